# revision 15
# baseline (speedup 1.0000x reference)
"""BitLinear forward on 8 Trainium2 NeuronCores.

Sharding: 2-way data parallel over tokens x 4-way tensor parallel over
output features. Each core computes y[token_half, out_quarter] =
[4096, 1024] of the full [8192, 4096] output.

Quantization uses an fp16 magic bias of 1536 = 1.5*2^10: for v in
[-130, 130], RN_f16(v + 1536) lands in [1024, 2048) where the fp16 ulp
is exactly 1, so the fp16 write itself rounds to the nearest integer
(ties-to-even, matching jnp.round). Activations are kept BIASED
(ab = a + 1536) through the fp16 matmul (same PE rate as bf16); the
constant bias is removed in the epilogue via y = psum - 1536*colsum(wq),
with colsum computed once per output chunk by a ones-vector matmul on
the PE. This needs one ACT pass per activation quarter instead of
three elementwise passes.

Per-core pipeline (all on-device):
  W phase (software-pipelined over 8 row-tiles): stream W rows, per row
    w_scale = mean|w| + eps (DVE reduces), ACT writes
    RN_f16(w*r + 1536), DVE clips to [1535,1537] and subtracts 1536
    (exact in fp16: small ints), DMA-transpose into wqT. w_scale*alpha
    and -384*colsum rows are bounced through DRAM scratch and
    broadcast-loaded.
  A phase: per 128-token tile, a_scale = max|x| + eps (DVE), one ACT
    pass per quarter writes biased bf16, DMA-transpose, then 32
    k-slice matmuls per 512-wide output chunk accumulate in PSUM.
  Epilogue (per chunk, drained immediately): DVE adds the -384*colsum
    broadcast row, then one fused scalar_tensor_tensor multiplies by
    stok (per-token) and w_scale*alpha (broadcast row), DMA out.

All matmul operands are exact integers in fp16 (|ab| <= 1663, ternary
weights), so the arithmetic is bit-exact in the f32 PSUM.
"""
import sys

sys.path.insert(0, "/opt/trn_rl_repo")

import numpy as np

B, S, DI, DOUT = 4, 2048, 4096, 4096
DP, TP = 2, 4
T_C = B * S // DP      # 4096 tokens per core
O_C = DOUT // TP       # 1024 out features per core
NT = T_C // 128        # 32 token tiles
NJ = O_C // 128        # 8 weight row tiles
QW = 1024              # quarter width along DI
NQ = DI // QW          # 4 quarters
KL = QW // 128         # 8 k-slices per quarter
OCW = 512              # output chunk width (one PSUM bank)
NOC = O_C // OCW       # 2 output chunks

EPS = 1e-8
QMAX = 127.0
MAGICB = 1536.0        # 1.5 * 2**10; fp16 add rounds to nearest-even int
MAGIC = 12582912.0     # 1.5 * 2**23; f32 add rounds to nearest-even int

_cached = {}


def _install_walrus_workarounds(tile_mod, mybir):
    """This walrus build rejects instructions with more than one sem wait
    ('Too many sync wait commands'). Split the Tile tail-drain waits over
    several sequencer drains; regular instructions are handled by
    _split_sync_waits after scheduling."""
    from concourse.vector_clock import ScopedClock

    def _drain_and_barrier_split(self, tick_clock, wait_clock):
        drain_inst = self.nc.sync.drain()
        wait_clock.add_sem_waits(
            drain_inst.ins, ScopedClock({None: tick_clock.global_clock})
        )
        waits = list(drain_inst.ins.sync_info.on_wait)
        if len(waits) > 1:
            del drain_inst.ins.sync_info.on_wait[1:]
            for w in waits[1:]:
                extra = self.nc.sync.drain()
                extra.ins.sync_info = mybir.SyncInfo(on_wait=[w], on_update=[])

        self.nc.all_engine_barrier()
        assert self.sems is not None
        popped = self.nc._tile_sem_poison_stack.pop()
        assert popped is self._sem_poison
        self.nc.clear_and_free_semaphores(list(self.sems.allocated().values()))
        self.nc.all_engine_barrier()

    tile_mod.TileContext._drain_and_barrier = _drain_and_barrier_split


def _split_sync_waits(nc, mybir, max_waits=1):
    """Move excess sem waits onto same-engine NoOps inserted before the
    offending instruction (engines run their stream in order, so the wait
    conjunction is preserved)."""
    n = 0
    for fn in nc.m.functions:
        for bb in fn.blocks:
            insts = bb.instructions
            i = 0
            while i < len(insts):
                inst = insts[i]
                si = getattr(inst, "sync_info", None)
                if si is not None and si.on_wait and len(si.on_wait) > max_waits:
                    waits = list(si.on_wait)
                    extra = waits[: len(waits) - max_waits]
                    del si.on_wait[: len(waits) - max_waits]
                    nops = []
                    for j in range(0, len(extra), max_waits):
                        nop = mybir.InstNoOp(name=f"WSPLIT-{n}", ins=[], outs=[])
                        n += 1
                        nop.engine = inst.engine
                        nop.sync_info = mybir.SyncInfo(
                            on_wait=list(extra[j : j + max_waits]), on_update=[]
                        )
                        nops.append(nop)
                    insts[i:i] = nops
                    i += len(nops)
                i += 1
    return n


def _build():
    import contextlib

    import concourse.bass as bass
    import concourse.tile as tile
    from concourse import mybir

    _install_walrus_workarounds(tile, mybir)

    F32 = mybir.dt.float32
    F16 = mybir.dt.float16
    Alu = mybir.AluOpType
    Act = mybir.ActivationFunctionType
    Ax = mybir.AxisListType

    nc = bass.Bass("TRN2", target_bir_lowering=False, debug=False, num_devices=8)
    x_d = nc.declare_dram_parameter("x", [T_C, DI], F32, isOutput=False)
    w_d = nc.declare_dram_parameter("w", [O_C, DI], F32, isOutput=False)
    al_d = nc.declare_dram_parameter("alpha", [O_C], F32, isOutput=False)
    y_d = nc.declare_dram_parameter("y", [T_C, O_C], F32, isOutput=True)
    scr_d = nc.dram_tensor("wsa_scratch", [O_C], F32)
    sc2_d = nc.dram_tensor("cs_scratch", [O_C], F32)

    with tile.TileContext(nc) as tc, contextlib.ExitStack() as ctx:
        xld = ctx.enter_context(tc.tile_pool(name="xld", bufs=6))
        wld = ctx.enter_context(tc.tile_pool(name="wld", bufs=6))
        aqp = ctx.enter_context(tc.tile_pool(name="aqp", bufs=2))
        wqp = ctx.enter_context(tc.tile_pool(name="wqp", bufs=2))
        tqp = ctx.enter_context(tc.tile_pool(name="tqp", bufs=5))
        wqt_p = ctx.enter_context(tc.tile_pool(name="wqt", bufs=1))
        bc_p = ctx.enter_context(tc.tile_pool(name="bc", bufs=1))
        sc = ctx.enter_context(tc.tile_pool(name="sc", bufs=8))
        sb_p = ctx.enter_context(tc.tile_pool(name="sb", bufs=4))
        ps = ctx.enter_context(tc.tile_pool(name="ps", bufs=3, space="PSUM"))
        cs_p = ctx.enter_context(tc.tile_pool(name="cs", bufs=1, space="PSUM"))

        onesk = bc_p.tile([128, 1], F16, tag="onesk")
        nc.vector.memset(onesk, 1.0)
        magica = bc_p.tile([128, 1], F32, tag="magica")
        nc.vector.memset(magica, MAGICB)
        posmagic = bc_p.tile([128, 1], F32, tag="posmagic")
        nc.vector.memset(posmagic, MAGIC)
        negmagic = bc_p.tile([128, 1], F32, tag="negmagic")
        nc.vector.memset(negmagic, -MAGIC)

        wqt = [
            wqt_p.tile([128, NQ * KL, OCW], F16, tag=f"wqt{oc}", name=f"wqt{oc}")
            for oc in range(NOC)
        ]
        bcast = [None] * NOC   # w_scale*alpha broadcast rows
        bcb = [None] * NOC     # -384*colsum broadcast rows
        wrows = {}             # j -> (parts, r) between W stages

        def emit_w_load(j):
            parts = []
            for qh in range(NQ):
                wt = wld.tile([128, QW], F32, tag="wld")
                nc.sync.dma_start(
                    out=wt, in_=w_d[j * 128:(j + 1) * 128, qh * QW:(qh + 1) * QW]
                )
                parts.append(wt)
            wrows[j] = parts

        def emit_w_stat(j):
            parts = wrows[j]
            ws4 = sc.tile([128, NQ], F32, tag="ws4")
            for qh in range(NQ):
                nc.vector.tensor_reduce(
                    out=ws4[:, qh:qh + 1], in_=parts[qh], axis=Ax.X, op=Alu.add,
                    apply_absolute_value=True,
                )
            tot = sc.tile([128, 1], F32, tag="wtot")
            nc.vector.tensor_reduce(out=tot, in_=ws4, axis=Ax.X, op=Alu.add)
            ws = sc.tile([128, 1], F32, tag="ws")
            nc.vector.tensor_scalar(
                out=ws, in0=tot, scalar1=1.0 / DI, scalar2=EPS,
                op0=Alu.mult, op1=Alu.add,
            )
            r = sc.tile([128, 1], F32, tag="wr")
            nc.vector.reciprocal(out=r, in_=ws)
            al_col = sc.tile([128, 1], F32, tag="al")
            nc.gpsimd.dma_start(
                out=al_col,
                in_=al_d[j * 128:(j + 1) * 128].rearrange("(o u) -> o u", u=1),
            )
            wsa = sc.tile([128, 1], F32, tag="wsa")
            nc.vector.tensor_tensor(out=wsa, in0=ws, in1=al_col, op=Alu.mult)
            nc.gpsimd.dma_start(
                out=bass.AP(tensor=scr_d, offset=j * 128, ap=[[1, 128]]),
                in_=wsa,
            )
            wrows[j] = (parts, r)

        def emit_w_quant(j):
            parts, r = wrows.pop(j)
            oc, jj = divmod(j, NJ // NOC)
            wq = wqp.tile([128, DI], F16, tag="wq")
            for qh in range(NQ):
                q = wq[:, qh * QW:(qh + 1) * QW]
                wt = parts[qh]
                # f32 magic add is a SINGLE rounding to the int grid (ulp=1
                # at 1.5*2^23), avoiding the double-round tie flips the fp16
                # magic would give on the weight side; clip while biased,
                # then ACT strips the bias on the fp16 write.
                nc.scalar.activation(
                    out=wt, in_=wt, func=Act.Identity,
                    bias=posmagic, scale=r,
                )
                nc.vector.tensor_scalar(
                    out=wt, in0=wt, scalar1=MAGIC + 1.0, scalar2=MAGIC - 1.0,
                    op0=Alu.min, op1=Alu.max,
                )
                nc.scalar.activation(
                    out=q, in_=wt, func=Act.Identity, bias=negmagic, scale=1.0,
                )
            nc.sync.dma_start_transpose(
                out=wqt[oc][:, :, jj * 128:(jj + 1) * 128], in_=wq
            )
            if jj == NJ // NOC - 1:
                bc = bc_p.tile([128, OCW], F32, tag=f"bc{oc}")
                nc.gpsimd.dma_start(
                    out=bc,
                    in_=bass.AP(
                        tensor=scr_d, offset=oc * OCW, ap=[[0, 128], [1, OCW]]
                    ),
                )
                bcast[oc] = bc

        def emit_colsum(oc):
            csp = cs_p.tile([1, OCW], F32, tag="cs", name=f"cs{oc}")
            for kk in range(NQ * KL):
                nc.tensor.matmul(
                    csp, lhsT=onesk, rhs=wqt[oc][:, kk, :],
                    start=(kk == 0), stop=(kk == NQ * KL - 1),
                )
            csr = bc_p.tile([1, OCW], F32, tag="csr")
            nc.scalar.activation(
                out=csr, in_=csp, func=Act.Copy, bias=0.0, scale=-MAGICB
            )
            nc.gpsimd.dma_start(
                out=bass.AP(tensor=sc2_d, offset=oc * OCW, ap=[[1, OCW]]),
                in_=csr.rearrange("u n -> (u n)"),
            )
            bb = bc_p.tile([128, OCW], F32, tag=f"bcb{oc}")
            nc.gpsimd.dma_start(
                out=bb,
                in_=bass.AP(
                    tensor=sc2_d, offset=oc * OCW, ap=[[0, 128], [1, OCW]]
                ),
            )
            bcb[oc] = bb

        def emit_a(t):
            parts = []
            for qh in range(NQ):
                xt = xld.tile([128, QW], F32, tag="xld")
                nc.sync.dma_start(
                    out=xt, in_=x_d[t * 128:(t + 1) * 128, qh * QW:(qh + 1) * QW]
                )
                parts.append(xt)
            am4 = sc.tile([128, NQ], F32, tag="am4")
            for qh in range(NQ):
                nc.vector.tensor_reduce(
                    out=am4[:, qh:qh + 1], in_=parts[qh], axis=Ax.X, op=Alu.max,
                    apply_absolute_value=True,
                )
            s = sc.tile([128, 1], F32, tag="s")
            nc.vector.tensor_reduce(out=s, in_=am4, axis=Ax.X, op=Alu.max)
            nc.vector.tensor_scalar_add(out=s, in0=s, scalar1=EPS)
            ra = sc.tile([128, 1], F32, tag="ra")
            nc.vector.reciprocal(out=ra, in_=s)
            i127 = sc.tile([128, 1], F32, tag="i127")
            nc.vector.tensor_scalar_mul(out=i127, in0=ra, scalar1=QMAX)
            stok = sc.tile([128, 1], F32, tag="stok")
            nc.vector.tensor_scalar_mul(out=stok, in0=s, scalar1=1.0 / QMAX)
            aqT = tqp.tile([128, NQ * KL, 128], F16, tag="aqT")
            aq = aqp.tile([128, DI], F16, tag="aq")
            for qh in range(NQ):
                nc.scalar.activation(
                    out=aq[:, qh * QW:(qh + 1) * QW], in_=parts[qh],
                    func=Act.Identity, bias=magica, scale=i127,
                )
            nc.scalar.dma_start_transpose(out=aqT, in_=aq)
            return aqT, stok

        def emit_mm(t, aqT, oc):
            psum = ps.tile([128, OCW], F32, tag=f"psum{oc}", name=f"ps{oc}_{t}")
            for kk in range(NQ * KL):
                nc.tensor.matmul(
                    psum,
                    lhsT=aqT[:, kk, :],
                    rhs=wqt[oc][:, kk, :],
                    start=(kk == 0),
                    stop=(kk == NQ * KL - 1),
                )
            return psum

        def emit_epi(t, oc, psum, stok):
            sb = sb_p.tile([128, OCW], F32, tag="sb")
            nc.vector.tensor_tensor(out=sb, in0=psum, in1=bcb[oc], op=Alu.add)
            nc.vector.scalar_tensor_tensor(
                out=sb, in0=sb, scalar=stok, in1=bcast[oc],
                op0=Alu.mult, op1=Alu.mult,
            )
            nc.gpsimd.dma_start(
                out=y_d[t * 128:(t + 1) * 128, oc * OCW:(oc + 1) * OCW],
                in_=sb,
            )

        # W rows 0..3 build wqt[0] (software-pipelined, loads run one row
        # ahead); colsum(0) right after so tile epilogues can drain from
        # tile 0. W rows 4..7 + colsum(1) interleave with tiles 0..3; oc1
        # chunks backlog in `pend` and drain at two per tile from t=4.
        NW0 = NJ // NOC  # 4: W tiles per output chunk
        emit_w_load(0)
        emit_w_load(1)
        emit_w_stat(0)
        emit_w_quant(0)
        emit_w_load(2)
        emit_w_stat(1)
        emit_w_quant(1)
        emit_w_load(3)
        emit_w_stat(2)
        emit_w_quant(2)
        emit_w_stat(3)
        emit_w_quant(3)
        emit_colsum(0)

        pend = []
        for t in range(NT):
            j = t + NW0
            if j < NJ:
                emit_w_load(j)
            aqT, stok = emit_a(t)
            psum = emit_mm(t, aqT, oc=0)
            emit_epi(t, 0, psum, stok)
            pend.append((t, aqT, stok))
            if j < NJ:
                emit_w_stat(j)
                emit_w_quant(j)
                if j == NJ - 1:
                    emit_colsum(1)
            else:
                for _ in range(2):
                    if pend:
                        tp, aT, st = pend.pop(0)
                        ps1 = emit_mm(tp, aT, oc=1)
                        emit_epi(tp, 1, ps1, st)
        while pend:
            tp, aT, st = pend.pop(0)
            ps1 = emit_mm(tp, aT, oc=1)
            emit_epi(tp, 1, ps1, st)

    n = _split_sync_waits(nc, mybir, max_waits=1)
    return nc


def _get_nc():
    if "nc" not in _cached:
        _cached["nc"] = _build()
    return _cached["nc"]


def _run(x, weight, alpha, trace=False):
    from concourse.bass_utils import run_bass_kernel_spmd

    nc = _get_nc()
    x_flat = np.ascontiguousarray(np.asarray(x).reshape(B * S, DI))
    weight = np.asarray(weight)
    alpha = np.asarray(alpha)
    in_maps = []
    for c in range(8):
        dp, tp = divmod(c, TP)
        in_maps.append(
            {
                "x": np.ascontiguousarray(x_flat[dp * T_C:(dp + 1) * T_C]),
                "w": np.ascontiguousarray(weight[tp * O_C:(tp + 1) * O_C]),
                "alpha": np.ascontiguousarray(alpha[tp * O_C:(tp + 1) * O_C]),
            }
        )
    res = run_bass_kernel_spmd(nc, in_maps, list(range(8)), trace=trace)
    y = np.empty((B * S, DOUT), np.float32)
    for c in range(8):
        dp, tp = divmod(c, TP)
        y[dp * T_C:(dp + 1) * T_C, tp * O_C:(tp + 1) * O_C] = res.results[c]["y"]
    return y.reshape(B, S, DOUT), res


def kernel(x, weight, alpha):
    y, _ = _run(x, weight, alpha, trace=False)
    return y
